# revision 29
# baseline (speedup 1.0000x reference)
"""Multi-head self-attention (B=2, T=2048, D=1024, H=16) on 8 TRN2 NeuronCores.

Sharding: batch x head-group. Core c handles batch b = c//4 and heads
h0 = 4*(c%4) .. h0+4 (Megatron-style column split of W_qkv, row split of
W_proj). Each core computes qkv projection for its heads, causal
flash-style attention for its 4 heads, and a partial output projection;
the host sums the 4 partial projections per batch (the Megatron
all-reduce realized as the unshard step) and adds b_proj.

Device algorithm (per core; all matmul operands bf16 -> 1 cyc/row at any
N, f32 PSUM accumulation):
  - qk^T[j, t] = sum_d W[d, j] x[t, d]  (j on partitions -> Q^T, K^T)
  - v[t, j]    = sum_d x[t, d] Wv[d, j] (t on partitions), packed into
    per-head [V_h|ones] / [ones|V_h] bf16 stationaries (parity-flipped so
    ctx lands on the lanes the ctxn head-pair packing needs)
  - S^T[kv, q] = K^T_tile.T @ Q^T, two heads row-packed per 2-bank PSUM
    pair; causal sub-diagonal tiles are skipped entirely, diagonal blocks
    masked by one strided DVE add of an inline tril constant
  - P^T = exp(S^T): one fused 2-head ScalarE activation per kv-tile,
    written bf16 (2x ACT throughput; softmax scale pre-folded into W_q)
  - one [ctx|den] matmul per (head, kv-tile) accumulates context and the
    softmax denominators together (denominator rows come from the ones
    half of the stationary, so they cost no extra PE cycles)
  - ctxn^T = ctx * reciprocal(den): reciprocal_approx_fast must run at
    partition base 0 (HW bug at base 64), so the recip rows are
    lane-shifted to the ctx lanes with a small SBUF->SBUF DMA
  - out_partial[t, e] = sum_f ctxn^T[f, t] Wp[f, e], written as bf16
    partials (host sums partials in fp32)

Scheduling: startup DMAs are split per-d-tile so the first qkv matmul
issues ~2us in (vs waiting for all weights); phases 2+3 run as ONE
global software pipeline: stage_b (ctx matmuls) trail stage_a (S^T +
exp) by DEPTH kv-tiles across ALL (qi, head-pair) block boundaries (no
drains), with the output-projection blocks of row-block qi interleaved
into the next block's stream. The PE queue therefore never idles, which
also keeps the tensor engine at its max p-state clock.
"""

import sys

if "/opt/trn_rl_repo" not in sys.path:
    sys.path.insert(0, "/opt/trn_rl_repo")

from collections import deque
from contextlib import ExitStack

import ml_dtypes
import numpy as np

import concourse.bass as bass
import concourse.bacc as bacc
import concourse.tile as tile
from concourse import mybir
from concourse.bass_utils import run_bass_kernel_spmd

B, T, D, H, DH = 2, 2048, 1024, 16, 64
NCORES = 8
HL = 4          # heads per core
P = 128         # SBUF/PSUM partitions
QT = 512        # q tile (moving free dim / PSUM bank)
KT = 128        # kv tile (PSUM partition dim)
F32 = mybir.dt.float32
F32R = mybir.dt.float32r
BF16 = mybir.dt.bfloat16


def _build_nc() -> bass.Bass:
    nc = bacc.Bacc(None)
    Exp = mybir.ActivationFunctionType.Exp
    Ident = mybir.ActivationFunctionType.Identity

    xT_d = nc.dram_tensor("xT", [D, T], BF16, kind="ExternalInput")
    wqk_d = nc.dram_tensor("wqk", [D, 512], BF16, kind="ExternalInput")
    wv_d = nc.dram_tensor("wv", [D, 256], BF16, kind="ExternalInput")
    bqk_d = nc.dram_tensor("bqk", [512], F32, kind="ExternalInput")
    bv_d = nc.dram_tensor("bv", [256], F32, kind="ExternalInput")
    wp_d = nc.dram_tensor("wp", [256, D], F32R, kind="ExternalInput")
    out_d = nc.dram_tensor("out", [T, D], BF16, kind="ExternalOutput")

    kv = np.arange(KT)
    tril_np = np.where(kv[:, None] <= kv[None, :], 0.0, -1e30).astype(np.float32)
    tril_d = nc.inline_tensor(tril_np, name="tril")

    with tile.TileContext(nc) as tc, ExitStack() as ctx:
        perm = ctx.enter_context(tc.tile_pool(name="perm", bufs=1))
        xpool = ctx.enter_context(tc.tile_pool(name="xpool", bufs=4))
        ppool = ctx.enter_context(tc.tile_pool(name="ppool", bufs=6))
        rpool = ctx.enter_context(tc.tile_pool(name="rpool", bufs=3))
        opool = ctx.enter_context(tc.tile_pool(name="opool", bufs=3))
        psA = ctx.enter_context(tc.tile_pool(name="psA", bufs=2, space="PSUM"))
        psC = ctx.enter_context(tc.tile_pool(name="psC", bufs=4, space="PSUM"))

        # --- the first x quarter rides the ACT HWDGE queue, split in chunks,
        # ahead of everything else on that queue, so the first qkv matmul can
        # issue ~2us in; later quarters prefetch on the (slow, idle) gpsimd
        # SWDGE queue ---
        xq0 = xpool.tile([P, 8, QT], BF16, name="xq0", tag="xq")
        for lo, hi in ((0, 1), (1, 2), (2, 4), (4, 6), (6, 8)):
            nc.scalar.dma_start(
                out=xq0[:, lo:hi, :],
                in_=xT_d[128 * lo:128 * hi, 0:QT].rearrange(
                    "(dt p) t -> p dt t", p=P
                ),
            )

        # --- weights / constants; DMAs split so the first matmul can start
        # as soon as the first (wqk, xq) chunk pair lands ---
        wqk_sb = perm.tile([P, 8, 512], BF16)
        for lo, hi in ((0, 1), (1, 2), (2, 4), (4, 6), (6, 8)):
            nc.sync.dma_start(
                out=wqk_sb[:, lo:hi, :],
                in_=wqk_d[128 * lo:128 * hi, :].rearrange(
                    "(dt p) j -> p dt j", p=P
                ),
            )
        wv_sb = perm.tile([P, 8, 256], BF16)
        for c in range(4):
            nc.sync.dma_start(
                out=wv_sb[:, 2 * c:2 * c + 2, :],
                in_=wv_d[256 * c:256 * (c + 1), :].rearrange(
                    "(dt p) j -> p dt j", p=P
                ),
            )
        # tril + bias consts ride the ACT queue so expwarm (which needs
        # tril) runs immediately and never blocks the phase-1 bias adds
        tril_sb = perm.tile([P, KT], F32)
        nc.scalar.dma_start(out=tril_sb, in_=tril_d[...])
        bqk_sb = perm.tile([P, 4], F32)
        nc.scalar.dma_start(out=bqk_sb, in_=bqk_d[...].rearrange("(jt p) -> p jt", p=P))
        bv_sb = perm.tile([P, 256], F32)
        nc.scalar.dma_start(
            out=bv_sb, in_=bass.AP(tensor=bv_d, offset=0, ap=[[0, P], [1, 256]])
        )
        wp_sb = perm.tile([P, 2, D], F32R)
        nc.sync.dma_start(
            out=wp_sb, in_=wp_d[...].rearrange("(ft p) e -> p ft e", p=P)
        )

        # first Exp triggers a ~2.7us ACT table load; fire it early on the
        # tiny tril const so it doesn't sit on the phase-1->2 critical chain
        expwarm = perm.tile([P, 8], F32)
        nc.scalar.activation(out=expwarm, in_=tril_sb[:, 0:8], func=Exp)

        qk_sb = perm.tile([P, 4, T], BF16)        # jt 0,1: Q^T; jt 2,3: K^T
        vaug_sb = perm.tile([P, 16, HL, 128], F32R)  # [kv, tt, h, V(64)|ones(64)]
        vaug_hh = vaug_sb.rearrange("p t (hp hh) c -> p t hp hh c", hh=2)
        ctxn_sb = perm.tile([P, 2, T], F32R)      # normalized ctx^T

        # the ones halves of the [V|ones]/[ones|V] stationaries are constant:
        # write them once up front (memset can't emit f32r, so write the f32
        # bit pattern of 1.0 through a uint32 view)
        ONE_F32 = 0x3F800000
        nc.vector.memset(
            vaug_hh[:, :, :, 0, 64:128].bitcast(mybir.dt.uint32), ONE_F32
        )
        nc.vector.memset(
            vaug_hh[:, :, :, 1, 0:64].bitcast(mybir.dt.uint32), ONE_F32
        )

        # --- phase-1 building blocks (emitted interleaved with attention) ---
        def qk_group(xq, tt4, jt):
            ps = psC.tile([P, QT], F32, name=f"qkps{tt4}_{jt}", tag="C")
            for dt in range(8):
                nc.tensor.matmul(
                    out=ps,
                    lhsT=wqk_sb[:, dt, jt * 128:(jt + 1) * 128],
                    rhs=xq[:, dt, :],
                    start=(dt == 0),
                    stop=(dt == 7),
                )
            nc.scalar.activation(
                out=qk_sb[:, jt, tt4 * QT:(tt4 + 1) * QT],
                in_=ps,
                func=Ident,
                bias=bqk_sb[:, jt:jt + 1],
                scale=1.0,
            )

        def v_group(xq, tt4, k):
            tt = tt4 * 4 + k
            psv = psA.tile([P, 256], F32, name=f"vps{tt}", tag="acc")
            for dt in range(8):
                nc.tensor.matmul(
                    out=psv,
                    lhsT=xq[:, dt, k * 128:(k + 1) * 128],
                    rhs=wv_sb[:, dt, :],
                    start=(dt == 0),
                    stop=(dt == 7),
                )
            # even heads fill [V|ones], odd heads fill [ones|V]; the flip
            # keeps ctx rows lane-aligned with the ctxn_sb head-pair packing
            vview = vaug_hh[:, tt]
            pview = psv.rearrange("p (hp hh d) -> p hp hh d", hp=2, hh=2)
            bview = bv_sb.rearrange("p (hp hh d) -> p hp hh d", hp=2, hh=2)
            nc.vector.tensor_add(
                out=vview[:, :, 0, 0:64], in0=pview[:, :, 0, :], in1=bview[:, :, 0, :]
            )
            nc.vector.tensor_add(
                out=vview[:, :, 1, 64:128], in0=pview[:, :, 1, :], in1=bview[:, :, 1, :]
            )

        DEPTH = 4
        Th_by_blk = {}

        def qoff_of(qi, j):
            return max(0, KT * j - qi * QT)

        def stage_a(hp, qi, j):
            q0 = qi * QT
            qoff = qoff_of(qi, j)
            s = psA.tile([P, 2 * QT], F32, name=f"s{hp}_{qi}_{j}", tag="acc")
            for hh in range(2):
                nc.tensor.matmul(
                    out=s[:, hh * QT + qoff: hh * QT + QT],
                    lhsT=qk_sb[hh * 64:(hh + 1) * 64, 2 + hp, j * KT:(j + 1) * KT],
                    rhs=qk_sb[hh * 64:(hh + 1) * 64, hp, q0 + qoff:q0 + QT],
                    start=True,
                    stop=True,
                )
            if j >= 4 * qi:
                # diagonal tile: mask the 128-wide triangular sub-block for
                # both heads with one strided add of the tril constant
                sm = s.rearrange("p (hh c) -> p hh c", hh=2)[:, :, qoff:qoff + KT]
                mask_b = bass.AP(
                    tensor=tril_sb.tensor,
                    offset=tril_sb.offset,
                    ap=[tril_sb.ap[0], [0, 2], [1, KT]],
                )
                nc.vector.tensor_add(out=sm, in0=sm, in1=mask_b)
            p_t = ppool.tile([P, 2, QT], F32R, name=f"p{hp}_{qi}_{j}", tag="p")
            sv = s.rearrange("p (hh c) -> p hh c", hh=2)
            nc.scalar.activation(
                out=p_t[:, :, qoff:QT], in_=sv[:, :, qoff:QT], func=Exp
            )
            return p_t

        def stage_b(hp, qi, j, p_t):
            njt = 4 * qi + 4
            qoff = qoff_of(qi, j)
            if j == 0:
                Th_by_blk[(hp, qi)] = [
                    psC.tile([P, QT], F32, name=f"T{hp}_{qi}_{hh}", tag="C")
                    for hh in range(2)
                ]
            Th = Th_by_blk[(hp, qi)]
            for hh in range(2):
                nc.tensor.matmul(
                    out=Th[hh][:, qoff:QT],
                    lhsT=vaug_sb[:, j, hp * 2 + hh, :],
                    rhs=p_t[:, hh, qoff:QT],
                    start=(j == 0),
                    stop=(j == njt - 1),
                )
            if j == njt - 1:
                normalize(hp, qi)

        def normalize(hp, qi):
            q0 = qi * QT
            Th = Th_by_blk.pop((hp, qi))
            for hh in range(2):
                cl = hh * 64          # ctx lanes base
                rec = rpool.tile([P, QT], F32, name=f"rec{hp}_{qi}_{hh}", tag="rec")
                # reciprocal_approx_fast mis-executes at partition base 64
                # (HW-verified), so always run it at base 0.
                if hh == 1:
                    nc.vector.reciprocal_approx_fast(out=rec[0:64, :], in_=Th[hh][0:64, :])
                    nc.sync.dma_start(out=rec[64:128, :], in_=rec[0:64, :])
                else:
                    nc.vector.tensor_copy(out=rec[64:128, :], in_=Th[hh][64:128, :])
                    nc.sync.dma_start(out=rec[0:64, :], in_=rec[64:128, :])
                    nc.vector.reciprocal_approx_fast(out=rec[0:64, :], in_=rec[0:64, :])
                nc.vector.tensor_mul(
                    out=ctxn_sb[cl:cl + 64, hp, q0:q0 + QT],
                    in0=Th[hh][cl:cl + 64, :],
                    in1=rec[cl:cl + 64, :],
                )

        def proj_block(tt):
            pj = psA.tile([P, 2, QT], F32, name=f"pj{tt}", tag="acc")
            ob = opool.tile([P, D], BF16, name=f"ob{tt}", tag="ob")
            for et in range(2):
                for ft in range(2):
                    nc.tensor.matmul(
                        out=pj[:, et, :],
                        lhsT=ctxn_sb[:, ft, tt * KT:(tt + 1) * KT],
                        rhs=wp_sb[:, ft, et * QT:(et + 1) * QT],
                        start=(ft == 0),
                        stop=(ft == 1),
                    )
            # PSUM->SBUF cast on DVE (ACT is the scarcer engine; GPSIMD
            # can't read PSUM)
            nc.vector.tensor_copy(out=ob, in_=pj.rearrange("p a b -> p (a b)"))
            nc.sync.dma_start(out=out_d[tt * KT:(tt + 1) * KT, :], in_=ob)

        # --- unified slot scheduler: qkv quarter q, then attention blocks
        # (q, hp) whose kv-tiles only need quarters <= q. Every PE work item
        # is a "slot"; pending ctx matmuls (stage_b) trail stage_a by DEPTH
        # slots and drain at 2/slot across block boundaries; proj blocks pop
        # a few slots after their row-block's last normalize. ---
        pend = deque()
        projq = deque()
        normed = {}
        slot = 0

        def emit_b():
            hp, qi, j, p_t = pend.popleft()
            stage_b(hp, qi, j, p_t)
            if j == 4 * qi + 3:
                normed[qi] = normed.get(qi, 0) + 1
                if normed[qi] == 2:
                    projq.extend((tt, slot) for tt in range(4 * qi, 4 * qi + 4))

        def tick(cur_block):
            nonlocal slot
            if pend and (pend[0][0], pend[0][1]) != cur_block:
                emit_b()
                if pend and (pend[0][0], pend[0][1]) != cur_block:
                    emit_b()
            elif len(pend) > DEPTH:
                emit_b()
            # proj blocks trail their qi's last normalize by >=3 slots so the
            # DVE reciprocal chain finishes before the PE reaches them
            if projq and slot % 2 == 0 and slot - projq[0][1] >= 3:
                proj_block(projq.popleft()[0])
            slot += 1

        # Phase A: K^T and V for every quarter (all kv-tiles ready), with the
        # next quarter's x prefetching on the gpsimd SWDGE queue. Phase B:
        # attention row-blocks in DESCENDING qi order, each preceded by its
        # own Q^T groups — the exp-heavy big blocks then interleave with Q
        # and proj PE filler, and the small blocks land at the tail. This
        # keeps ScalarE's exp stream below the PE's slot rate everywhere.
        xq_tiles = {0: xq0}
        for q in range(T // QT):
            if q + 1 < T // QT:
                nxt = xpool.tile([P, 8, QT], BF16, name=f"xq{q + 1}", tag="xq")
                nc.gpsimd.dma_start(
                    out=nxt,
                    in_=xT_d[:, (q + 1) * QT:(q + 2) * QT].rearrange(
                        "(dt p) t -> p dt t", p=P
                    ),
                )
                xq_tiles[q + 1] = nxt
            for jt in (2, 3):
                qk_group(xq_tiles[q], q, jt)
                tick(None)
            for k in range(4):
                v_group(xq_tiles[q], q, k)
                tick(None)
        for qi in range(T // QT - 1, -1, -1):
            for jt in (0, 1):
                qk_group(xq_tiles[qi], qi, jt)
                tick(None)
            for hp in range(2):
                for j in range(4 * qi + 4):
                    p_t = stage_a(hp, qi, j)
                    pend.append((hp, qi, j, p_t))
                    tick((hp, qi))
        while pend:
            emit_b()
        while projq:
            proj_block(projq.popleft()[0])

    nc.finalize()
    return nc


_NC_CACHE: list = []


def _get_nc() -> bass.Bass:
    if not _NC_CACHE:
        _NC_CACHE.append(_build_nc())
    return _NC_CACHE[0]


def _shard_inputs(x, W_qkv, b_qkv, W_proj):
    scale = np.float32(1.0 / np.sqrt(DH))
    bf16 = ml_dtypes.bfloat16
    in_maps = []
    xTs = [np.ascontiguousarray(x[b].T.astype(bf16)) for b in range(B)]
    for c in range(NCORES):
        b = c // 4
        lo = (c % 4) * HL * DH
        wqk = np.concatenate(
            [W_qkv[:, lo:lo + 256] * scale, W_qkv[:, D + lo:D + lo + 256]], axis=1
        )
        bqk = np.concatenate([b_qkv[lo:lo + 256] * scale, b_qkv[D + lo:D + lo + 256]])
        in_maps.append({
            "xT": xTs[b],
            "wqk": np.ascontiguousarray(wqk.astype(bf16)),
            "wv": np.ascontiguousarray(W_qkv[:, 2 * D + lo:2 * D + lo + 256].astype(bf16)),
            "bqk": np.ascontiguousarray(bqk, np.float32),
            "bv": np.ascontiguousarray(b_qkv[2 * D + lo:2 * D + lo + 256], np.float32),
            "wp": np.ascontiguousarray(W_proj[lo:lo + 256, :], np.float32),
        })
    return in_maps


def kernel(x, W_qkv, b_qkv, W_proj, b_proj, _trace=False, _tmpdir=None):
    x = np.asarray(x, np.float32)
    W_qkv = np.asarray(W_qkv, np.float32)
    b_qkv = np.asarray(b_qkv, np.float32)
    W_proj = np.asarray(W_proj, np.float32)
    b_proj = np.asarray(b_proj, np.float32)

    nc = _get_nc()
    in_maps = _shard_inputs(x, W_qkv, b_qkv, W_proj)
    kw = {}
    if _trace:
        kw = dict(trace=True, tmpdir=_tmpdir)
    r = run_bass_kernel_spmd(nc, in_maps, core_ids=list(range(NCORES)), **kw)
    out = np.zeros((B, T, D), np.float32)
    for c in range(NCORES):
        out[c // 4] += np.asarray(r.results[c]["out"], np.float32)
    out += b_proj[None, None, :]
    if _trace:
        return out, r
    return out


# revision 36
# speedup vs baseline: 1.0573x; 1.0573x over previous
"""Multi-head self-attention (B=2, T=2048, D=1024, H=16) on 8 TRN2 NeuronCores.

Sharding: batch x head-group. Core c handles batch b = c//4 and heads
h0 = 4*(c%4) .. h0+4 (Megatron-style column split of W_qkv, row split of
W_proj). Each core computes qkv projection for its heads, causal
flash-style attention for its 4 heads, and a partial output projection;
the host sums the 4 partial projections per batch (the Megatron
all-reduce realized as the unshard step) and adds b_proj.

Device algorithm (per core; all matmul operands bf16 -> 1 cyc/row at any
N, f32 PSUM accumulation):
  - qk^T[j, t] = sum_d W[d, j] x[t, d]  (j on partitions -> Q^T, K^T)
  - v[t, j]    = sum_d x[t, d] Wv[d, j] (t on partitions), packed into
    per-head [V_h|ones] / [ones|V_h] bf16 stationaries (parity-flipped so
    ctx lands on the lanes the ctxn head-pair packing needs)
  - S^T[kv, q] = K^T_tile.T @ Q^T, two heads row-packed per 2-bank PSUM
    pair; causal sub-diagonal tiles are skipped entirely, diagonal blocks
    masked by one strided DVE add of an inline tril constant
  - P^T = exp(S^T): one fused 2-head ScalarE activation per kv-tile,
    written bf16 (2x ACT throughput; softmax scale pre-folded into W_q)
  - one [ctx|den] matmul per (head, kv-tile) accumulates context and the
    softmax denominators together (denominator rows come from the ones
    half of the stationary, so they cost no extra PE cycles)
  - ctxn^T = ctx * reciprocal(den): reciprocal_approx_fast must run at
    partition base 0 (HW bug at base 64), so the recip rows are
    lane-shifted to the ctx lanes with a small SBUF->SBUF DMA
  - out_partial[t, e] = sum_f ctxn^T[f, t] Wp[f, e], written as bf16
    partials (host sums partials in fp32)

Scheduling: startup DMAs are split per-d-tile so the first qkv matmul
issues ~2us in (vs waiting for all weights); phases 2+3 run as ONE
global software pipeline: stage_b (ctx matmuls) trail stage_a (S^T +
exp) by DEPTH kv-tiles across ALL (qi, head-pair) block boundaries (no
drains), with the output-projection blocks of row-block qi interleaved
into the next block's stream. The PE queue therefore never idles, which
also keeps the tensor engine at its max p-state clock.
"""

import sys

if "/opt/trn_rl_repo" not in sys.path:
    sys.path.insert(0, "/opt/trn_rl_repo")

from collections import deque
from contextlib import ExitStack

import ml_dtypes
import numpy as np

import concourse.bass as bass
import concourse.bacc as bacc
import concourse.tile as tile
from concourse import mybir
from concourse.bass_utils import run_bass_kernel_spmd

B, T, D, H, DH = 2, 2048, 1024, 16, 64
NCORES = 8
HL = 4          # heads per core
P = 128         # SBUF/PSUM partitions
QT = 512        # q tile (moving free dim / PSUM bank)
KT = 128        # kv tile (PSUM partition dim)
F32 = mybir.dt.float32
F32R = mybir.dt.float32r
BF16 = mybir.dt.bfloat16


def _build_nc() -> bass.Bass:
    nc = bacc.Bacc(None)
    Exp = mybir.ActivationFunctionType.Exp
    Ident = mybir.ActivationFunctionType.Identity

    xT_d = nc.dram_tensor("xT", [D, T], BF16, kind="ExternalInput")
    wqk_d = nc.dram_tensor("wqk", [D, 512], BF16, kind="ExternalInput")
    wv_d = nc.dram_tensor("wv", [D, 256], BF16, kind="ExternalInput")
    bqk_d = nc.dram_tensor("bqk", [512], F32, kind="ExternalInput")
    bv_d = nc.dram_tensor("bv", [256], F32, kind="ExternalInput")
    wp_d = nc.dram_tensor("wp", [256, D], F32R, kind="ExternalInput")
    out_d = nc.dram_tensor("out", [T, D], BF16, kind="ExternalOutput")

    kv = np.arange(KT)
    tril_np = np.where(kv[:, None] <= kv[None, :], 0.0, -1e30).astype(np.float32)
    tril_d = nc.inline_tensor(tril_np, name="tril")
    trilb_np = tril_np.astype(ml_dtypes.bfloat16)
    trilb_d = nc.inline_tensor(trilb_np, name="trilb")
    eye_np = np.eye(KT, dtype=ml_dtypes.bfloat16)
    eye_d = nc.inline_tensor(eye_np, name="eye")

    with tile.TileContext(nc) as tc, ExitStack() as ctx:
        perm = ctx.enter_context(tc.tile_pool(name="perm", bufs=1))
        xpool = ctx.enter_context(tc.tile_pool(name="xpool", bufs=4))
        ppool = ctx.enter_context(tc.tile_pool(name="ppool", bufs=6))
        rpool = ctx.enter_context(tc.tile_pool(name="rpool", bufs=3))
        opool = ctx.enter_context(tc.tile_pool(name="opool", bufs=3))
        psA = ctx.enter_context(tc.tile_pool(name="psA", bufs=2, space="PSUM"))
        psC = ctx.enter_context(tc.tile_pool(name="psC", bufs=4, space="PSUM"))

        # --- the first x quarter rides the ACT HWDGE queue, split in chunks,
        # ahead of everything else on that queue, so the first qkv matmul can
        # issue ~2us in; later quarters prefetch on the (slow, idle) gpsimd
        # SWDGE queue ---
        xq0 = xpool.tile([P, 8, QT], BF16, name="xq0", tag="xq")
        for lo, hi in ((0, 1), (1, 2), (2, 4), (4, 6), (6, 8)):
            nc.scalar.dma_start(
                out=xq0[:, lo:hi, :],
                in_=xT_d[128 * lo:128 * hi, 0:QT].rearrange(
                    "(dt p) t -> p dt t", p=P
                ),
            )

        # --- weights / constants; DMAs split so the first matmul can start
        # as soon as the first (wqk, xq) chunk pair lands ---
        wqk_sb = perm.tile([P, 8, 512], BF16)
        for lo, hi in ((0, 1), (1, 2), (2, 4), (4, 6), (6, 8)):
            nc.sync.dma_start(
                out=wqk_sb[:, lo:hi, :],
                in_=wqk_d[128 * lo:128 * hi, :].rearrange(
                    "(dt p) j -> p dt j", p=P
                ),
            )
        wv_sb = perm.tile([P, 8, 256], BF16)
        for c in range(4):
            nc.sync.dma_start(
                out=wv_sb[:, 2 * c:2 * c + 2, :],
                in_=wv_d[256 * c:256 * (c + 1), :].rearrange(
                    "(dt p) j -> p dt j", p=P
                ),
            )
        # tril + bias consts ride the ACT queue so expwarm (which needs
        # tril) runs immediately and never blocks the phase-1 bias adds
        tril_sb = perm.tile([P, KT], F32)
        nc.scalar.dma_start(out=tril_sb, in_=tril_d[...])
        trilb_sb = perm.tile([P, KT], BF16)
        nc.scalar.dma_start(out=trilb_sb, in_=trilb_d[...])
        eye_sb = perm.tile([P, KT], BF16)
        nc.scalar.dma_start(out=eye_sb, in_=eye_d[...])
        bqk_sb = perm.tile([P, 4], F32)
        nc.scalar.dma_start(out=bqk_sb, in_=bqk_d[...].rearrange("(jt p) -> p jt", p=P))
        bv_sb = perm.tile([P, 256], F32)
        nc.scalar.dma_start(
            out=bv_sb, in_=bass.AP(tensor=bv_d, offset=0, ap=[[0, P], [1, 256]])
        )
        wp_sb = perm.tile([P, 2, D], F32R)
        nc.sync.dma_start(
            out=wp_sb, in_=wp_d[...].rearrange("(ft p) e -> p ft e", p=P)
        )

        # first Exp triggers a ~2.7us ACT table load; fire it early on the
        # tiny tril const so it doesn't sit on the phase-1->2 critical chain
        expwarm = perm.tile([P, 8], F32)
        nc.scalar.activation(out=expwarm, in_=tril_sb[:, 0:8], func=Exp)

        qk_sb = perm.tile([P, 4, T], BF16)        # jt 0,1: Q^T; jt 2,3: K^T
        vaug_sb = perm.tile([P, 16, HL, 128], F32R)  # [kv, tt, h, V(64)|ones(64)]
        vaug_hh = vaug_sb.rearrange("p t (hp hh) c -> p t hp hh c", hh=2)
        ctxn_sb = perm.tile([P, 2, T], F32R)      # normalized ctx^T

        # the ones halves of the [V|ones]/[ones|V] stationaries are constant:
        # write them once up front (memset can't emit f32r, so write the f32
        # bit pattern of 1.0 through a uint32 view)
        ONE_F32 = 0x3F800000
        nc.gpsimd.memset(
            vaug_hh[:, :, :, 0, 64:128].bitcast(mybir.dt.uint32), ONE_F32
        )
        nc.gpsimd.memset(
            vaug_hh[:, :, :, 1, 0:64].bitcast(mybir.dt.uint32), ONE_F32
        )

        # --- phase-1 building blocks (emitted interleaved with attention) ---
        def qk_group(xq, tt4, jt):
            ps = psC.tile([P, QT], F32, name=f"qkps{tt4}_{jt}", tag="C")
            for dt in range(8):
                nc.tensor.matmul(
                    out=ps,
                    lhsT=wqk_sb[:, dt, jt * 128:(jt + 1) * 128],
                    rhs=xq[:, dt, :],
                    start=(dt == 0),
                    stop=(dt == 7),
                )
            nc.scalar.activation(
                out=qk_sb[:, jt, tt4 * QT:(tt4 + 1) * QT],
                in_=ps,
                func=Ident,
                bias=bqk_sb[:, jt:jt + 1],
                scale=1.0,
            )

        def v_group(xq, tt4, k):
            tt = tt4 * 4 + k
            psv = psA.tile([P, 256], F32, name=f"vps{tt}", tag="acc")
            for dt in range(8):
                nc.tensor.matmul(
                    out=psv,
                    lhsT=xq[:, dt, k * 128:(k + 1) * 128],
                    rhs=wv_sb[:, dt, :],
                    start=(dt == 0),
                    stop=(dt == 7),
                )
            # even heads fill [V|ones], odd heads fill [ones|V]; the flip
            # keeps ctx rows lane-aligned with the ctxn_sb head-pair packing
            vview = vaug_hh[:, tt]
            pview = psv.rearrange("p (hp hh d) -> p hp hh d", hp=2, hh=2)
            bview = bv_sb.rearrange("p (hp hh d) -> p hp hh d", hp=2, hh=2)
            nc.vector.tensor_add(
                out=vview[:, :, 0, 0:64], in0=pview[:, :, 0, :], in1=bview[:, :, 0, :]
            )
            nc.vector.tensor_add(
                out=vview[:, :, 1, 64:128], in0=pview[:, :, 1, :], in1=bview[:, :, 1, :]
            )

        DEPTH = 4
        Th_by_blk = {}

        def qoff_of(qi, j):
            return max(0, KT * j - qi * QT)

        def stage_a(hp, qi, j):
            q0 = qi * QT
            qoff = qoff_of(qi, j)
            masked = j >= 4 * qi
            s = psA.tile([P, 2 * QT], F32, name=f"s{hp}_{qi}_{j}", tag="acc")
            for hh in range(2):
                nc.tensor.matmul(
                    out=s[:, hh * QT + qoff: hh * QT + QT],
                    lhsT=qk_sb[hh * 64:(hh + 1) * 64, 2 + hp, j * KT:(j + 1) * KT],
                    rhs=qk_sb[hh * 64:(hh + 1) * 64, hp, q0 + qoff:q0 + QT],
                    start=True,
                    stop=not masked,
                )
            if masked:
                # diagonal tile: accumulate the -1e30 tril constant into the
                # 128-wide triangular sub-block ON THE PE (identity
                # stationary, broadcast moving) — keeps the S -> exp chain
                # off the DVE queue entirely
                sm = s.rearrange("p (hh c) -> p hh c", hh=2)[:, :, qoff:qoff + KT]
                mask_b = bass.AP(
                    tensor=trilb_sb.tensor,
                    offset=trilb_sb.offset,
                    ap=[trilb_sb.ap[0], [0, 2], [1, KT]],
                )
                nc.tensor.matmul(
                    out=sm, lhsT=eye_sb, rhs=mask_b, start=False, stop=True,
                )
            p_t = ppool.tile([P, 2, QT], F32R, name=f"p{hp}_{qi}_{j}", tag="p")
            sv = s.rearrange("p (hh c) -> p hh c", hh=2)
            nc.scalar.activation(
                out=p_t[:, :, qoff:QT], in_=sv[:, :, qoff:QT], func=Exp
            )
            return p_t

        def stage_b(hp, qi, j, p_t):
            njt = 4 * qi + 4
            qoff = qoff_of(qi, j)
            if j == 0:
                Th_by_blk[(hp, qi)] = [
                    psC.tile([P, QT], F32, name=f"T{hp}_{qi}_{hh}", tag="C")
                    for hh in range(2)
                ]
            Th = Th_by_blk[(hp, qi)]
            for hh in range(2):
                nc.tensor.matmul(
                    out=Th[hh][:, qoff:QT],
                    lhsT=vaug_sb[:, j, hp * 2 + hh, :],
                    rhs=p_t[:, hh, qoff:QT],
                    start=(j == 0),
                    stop=(j == njt - 1),
                )
            if j == njt - 1:
                normalize(hp, qi)

        def normalize(hp, qi):
            q0 = qi * QT
            Th = Th_by_blk.pop((hp, qi))
            for hh in range(2):
                cl = hh * 64          # ctx lanes base
                rec = rpool.tile([P, QT], F32, name=f"rec{hp}_{qi}_{hh}", tag="rec")
                # reciprocal_approx_fast mis-executes at partition base 64
                # (HW-verified), so always run it at base 0.
                if hh == 1:
                    nc.vector.reciprocal_approx_fast(out=rec[0:64, :], in_=Th[hh][0:64, :])
                    nc.sync.dma_start(out=rec[64:128, :], in_=rec[0:64, :])
                else:
                    # the den-row copy rides ACT (DVE is the scarcer engine
                    # in the normalize chain)
                    nc.scalar.activation(
                        out=rec[64:128, :], in_=Th[hh][64:128, :], func=Ident
                    )
                    nc.sync.dma_start(out=rec[0:64, :], in_=rec[64:128, :])
                    nc.vector.reciprocal_approx_fast(out=rec[0:64, :], in_=rec[0:64, :])
                nc.vector.tensor_mul(
                    out=ctxn_sb[cl:cl + 64, hp, q0:q0 + QT],
                    in0=Th[hh][cl:cl + 64, :],
                    in1=rec[cl:cl + 64, :],
                )

        def proj_block(tt):
            pj = psA.tile([P, 2, QT], F32, name=f"pj{tt}", tag="acc")
            ob = opool.tile([P, D], BF16, name=f"ob{tt}", tag="ob")
            for et in range(2):
                for ft in range(2):
                    nc.tensor.matmul(
                        out=pj[:, et, :],
                        lhsT=ctxn_sb[:, ft, tt * KT:(tt + 1) * KT],
                        rhs=wp_sb[:, ft, et * QT:(et + 1) * QT],
                        start=(ft == 0),
                        stop=(ft == 1),
                    )
            # PSUM->SBUF cast on DVE (ACT is the scarcer engine; GPSIMD
            # can't read PSUM)
            nc.vector.tensor_copy(out=ob, in_=pj.rearrange("p a b -> p (a b)"))
            nc.sync.dma_start(out=out_d[tt * KT:(tt + 1) * KT, :], in_=ob)

        # --- unified slot scheduler: qkv quarter q, then attention blocks
        # (q, hp) whose kv-tiles only need quarters <= q. Every PE work item
        # is a "slot"; pending ctx matmuls (stage_b) trail stage_a by DEPTH
        # slots and drain at 2/slot across block boundaries; proj blocks pop
        # a few slots after their row-block's last normalize. ---
        pend = deque()
        projq = deque()
        normed = {}
        slot = 0

        def emit_b():
            hp, qi, j, p_t = pend.popleft()
            stage_b(hp, qi, j, p_t)
            if j == 4 * qi + 3:
                normed[qi] = normed.get(qi, 0) + 1
                if normed[qi] == 2:
                    projq.extend((tt, slot) for tt in range(4 * qi, 4 * qi + 4))

        def tick(cur_block):
            nonlocal slot
            if pend and (pend[0][0], pend[0][1]) != cur_block:
                emit_b()
                if pend and (pend[0][0], pend[0][1]) != cur_block:
                    emit_b()
            elif len(pend) > DEPTH:
                emit_b()
            # proj blocks trail their qi's last normalize by >=4 slots so the
            # DVE reciprocal chain finishes before the PE reaches them
            if projq and slot % 2 == 0 and slot - projq[0][1] >= 4:
                proj_block(projq.popleft()[0])
            slot += 1

        xq_tiles = {0: xq0}
        for q in range(T // QT):
            # prefetch the NEXT quarter's x on the idle gpsimd SWDGE queue
            if q + 1 < T // QT:
                nxt = xpool.tile([P, 8, QT], BF16, name=f"xq{q + 1}", tag="xq")
                nc.gpsimd.dma_start(
                    out=nxt,
                    in_=xT_d[:, (q + 1) * QT:(q + 2) * QT].rearrange(
                        "(dt p) t -> p dt t", p=P
                    ),
                )
                xq_tiles[q + 1] = nxt
            xq = xq_tiles.pop(q)
            for jt in range(4):
                qk_group(xq, q, jt)
                tick(None)
            for k in range(4):
                v_group(xq, q, k)
                tick(None)
            for hp in range(2):
                for j in range(4 * q + 4):
                    p_t = stage_a(hp, q, j)
                    pend.append((hp, q, j, p_t))
                    tick((hp, q))
        while pend:
            emit_b()
        while projq:
            proj_block(projq.popleft()[0])

    nc.finalize()
    return nc


_NC_CACHE: list = []


def _get_nc() -> bass.Bass:
    if not _NC_CACHE:
        _NC_CACHE.append(_build_nc())
    return _NC_CACHE[0]


def _shard_inputs(x, W_qkv, b_qkv, W_proj):
    scale = np.float32(1.0 / np.sqrt(DH))
    bf16 = ml_dtypes.bfloat16
    in_maps = []
    xTs = [np.ascontiguousarray(x[b].T.astype(bf16)) for b in range(B)]
    for c in range(NCORES):
        b = c // 4
        lo = (c % 4) * HL * DH
        wqk = np.concatenate(
            [W_qkv[:, lo:lo + 256] * scale, W_qkv[:, D + lo:D + lo + 256]], axis=1
        )
        bqk = np.concatenate([b_qkv[lo:lo + 256] * scale, b_qkv[D + lo:D + lo + 256]])
        in_maps.append({
            "xT": xTs[b],
            "wqk": np.ascontiguousarray(wqk.astype(bf16)),
            "wv": np.ascontiguousarray(W_qkv[:, 2 * D + lo:2 * D + lo + 256].astype(bf16)),
            "bqk": np.ascontiguousarray(bqk, np.float32),
            "bv": np.ascontiguousarray(b_qkv[2 * D + lo:2 * D + lo + 256], np.float32),
            "wp": np.ascontiguousarray(W_proj[lo:lo + 256, :], np.float32),
        })
    return in_maps


def kernel(x, W_qkv, b_qkv, W_proj, b_proj, _trace=False, _tmpdir=None):
    x = np.asarray(x, np.float32)
    W_qkv = np.asarray(W_qkv, np.float32)
    b_qkv = np.asarray(b_qkv, np.float32)
    W_proj = np.asarray(W_proj, np.float32)
    b_proj = np.asarray(b_proj, np.float32)

    nc = _get_nc()
    in_maps = _shard_inputs(x, W_qkv, b_qkv, W_proj)
    kw = {}
    if _trace:
        kw = dict(trace=True, tmpdir=_tmpdir)
    r = run_bass_kernel_spmd(nc, in_maps, core_ids=list(range(NCORES)), **kw)
    out = np.zeros((B, T, D), np.float32)
    for c in range(NCORES):
        out[c // 4] += np.asarray(r.results[c]["out"], np.float32)
    out += b_proj[None, None, :]
    if _trace:
        return out, r
    return out


# revision 38
# speedup vs baseline: 1.0757x; 1.0175x over previous
"""Multi-head self-attention (B=2, T=2048, D=1024, H=16) on 8 TRN2 NeuronCores.

Sharding: batch x head-group. Core c handles batch b = c//4 and heads
h0 = 4*(c%4) .. h0+4 (Megatron-style column split of W_qkv, row split of
W_proj). Each core computes qkv projection for its heads, causal
flash-style attention for its 4 heads, and a partial output projection;
the host sums the 4 partial projections per batch (the Megatron
all-reduce realized as the unshard step) and adds b_proj.

Device algorithm (per core; all matmul operands bf16 -> 1 cyc/row at any
N, f32 PSUM accumulation):
  - qk^T[j, t] = sum_d W[d, j] x[t, d]  (j on partitions -> Q^T, K^T)
  - v[t, j]    = sum_d x[t, d] Wv[d, j] (t on partitions), packed into
    per-head [V_h|ones] / [ones|V_h] bf16 stationaries (parity-flipped so
    ctx lands on the lanes the ctxn head-pair packing needs)
  - S^T[kv, q] = K^T_tile.T @ Q^T, two heads row-packed per 2-bank PSUM
    pair; causal sub-diagonal tiles are skipped entirely, diagonal blocks
    masked by one strided DVE add of an inline tril constant
  - P^T = exp(S^T): one fused 2-head ScalarE activation per kv-tile,
    written bf16 (2x ACT throughput; softmax scale pre-folded into W_q)
  - one [ctx|den] matmul per (head, kv-tile) accumulates context and the
    softmax denominators together (denominator rows come from the ones
    half of the stationary, so they cost no extra PE cycles)
  - ctxn^T = ctx * reciprocal(den): reciprocal_approx_fast must run at
    partition base 0 (HW bug at base 64), so the recip rows are
    lane-shifted to the ctx lanes with a small SBUF->SBUF DMA
  - out_partial[t, e] = sum_f ctxn^T[f, t] Wp[f, e], written as bf16
    partials (host sums partials in fp32)

Scheduling: startup DMAs are split per-d-tile so the first qkv matmul
issues ~2us in (vs waiting for all weights); phases 2+3 run as ONE
global software pipeline: stage_b (ctx matmuls) trail stage_a (S^T +
exp) by DEPTH kv-tiles across ALL (qi, head-pair) block boundaries (no
drains), with the output-projection blocks of row-block qi interleaved
into the next block's stream. The PE queue therefore never idles, which
also keeps the tensor engine at its max p-state clock.
"""

import sys

if "/opt/trn_rl_repo" not in sys.path:
    sys.path.insert(0, "/opt/trn_rl_repo")

from collections import deque
from contextlib import ExitStack

import ml_dtypes
import numpy as np

import concourse.bass as bass
import concourse.bacc as bacc
import concourse.tile as tile
from concourse import mybir
from concourse.bass_utils import run_bass_kernel_spmd

B, T, D, H, DH = 2, 2048, 1024, 16, 64
NCORES = 8
HL = 4          # heads per core
P = 128         # SBUF/PSUM partitions
QT = 512        # q tile (moving free dim / PSUM bank)
KT = 128        # kv tile (PSUM partition dim)
F32 = mybir.dt.float32
F32R = mybir.dt.float32r
BF16 = mybir.dt.bfloat16


def _build_nc() -> bass.Bass:
    nc = bacc.Bacc(None)
    Exp = mybir.ActivationFunctionType.Exp
    Ident = mybir.ActivationFunctionType.Identity

    xT_d = nc.dram_tensor("xT", [D, T], BF16, kind="ExternalInput")
    wqk_d = nc.dram_tensor("wqk", [D, 512], BF16, kind="ExternalInput")
    wv_d = nc.dram_tensor("wv", [D, 256], BF16, kind="ExternalInput")
    bqk_d = nc.dram_tensor("bqk", [512], F32, kind="ExternalInput")
    bv_d = nc.dram_tensor("bv", [256], F32, kind="ExternalInput")
    wp_d = nc.dram_tensor("wp", [256, D], F32R, kind="ExternalInput")
    out_d = nc.dram_tensor("out", [T, D], BF16, kind="ExternalOutput")

    kv = np.arange(KT)
    tril_np = np.where(kv[:, None] <= kv[None, :], 0.0, -1e30).astype(np.float32)
    tril_d = nc.inline_tensor(tril_np, name="tril")
    trilb_np = tril_np.astype(ml_dtypes.bfloat16)
    trilb_d = nc.inline_tensor(trilb_np, name="trilb")
    eye_np = np.eye(KT, dtype=ml_dtypes.bfloat16)
    eye_d = nc.inline_tensor(eye_np, name="eye")

    with tile.TileContext(nc) as tc, ExitStack() as ctx:
        perm = ctx.enter_context(tc.tile_pool(name="perm", bufs=1))
        xpool = ctx.enter_context(tc.tile_pool(name="xpool", bufs=4))
        ppool = ctx.enter_context(tc.tile_pool(name="ppool", bufs=6))
        rpool = ctx.enter_context(tc.tile_pool(name="rpool", bufs=3))
        opool = ctx.enter_context(tc.tile_pool(name="opool", bufs=3))
        psA = ctx.enter_context(tc.tile_pool(name="psA", bufs=2, space="PSUM"))
        psC = ctx.enter_context(tc.tile_pool(name="psC", bufs=4, space="PSUM"))

        # --- the first x quarter rides the ACT HWDGE queue, split in chunks,
        # ahead of everything else on that queue, so the first qkv matmul can
        # issue ~2us in; later quarters prefetch on the (slow, idle) gpsimd
        # SWDGE queue ---
        xq0 = xpool.tile([P, 8, QT], BF16, name="xq0", tag="xq")
        for lo, hi in ((0, 1), (1, 2), (2, 4), (4, 6), (6, 8)):
            nc.scalar.dma_start(
                out=xq0[:, lo:hi, :],
                in_=xT_d[128 * lo:128 * hi, 0:QT].rearrange(
                    "(dt p) t -> p dt t", p=P
                ),
            )

        # --- weights / constants; DMAs split so the first matmul can start
        # as soon as the first (wqk, xq) chunk pair lands ---
        wqk_sb = perm.tile([P, 8, 512], BF16)
        for lo, hi in ((0, 1), (1, 2), (2, 4), (4, 6), (6, 8)):
            nc.sync.dma_start(
                out=wqk_sb[:, lo:hi, :],
                in_=wqk_d[128 * lo:128 * hi, :].rearrange(
                    "(dt p) j -> p dt j", p=P
                ),
            )
        wv_sb = perm.tile([P, 8, 256], BF16)
        for c in range(4):
            nc.sync.dma_start(
                out=wv_sb[:, 2 * c:2 * c + 2, :],
                in_=wv_d[256 * c:256 * (c + 1), :].rearrange(
                    "(dt p) j -> p dt j", p=P
                ),
            )
        # tril + bias consts ride the ACT queue so expwarm (which needs
        # tril) runs immediately and never blocks the phase-1 bias adds
        tril_sb = perm.tile([P, KT], F32)
        nc.scalar.dma_start(out=tril_sb, in_=tril_d[...])
        trilb_sb = perm.tile([P, KT], BF16)
        nc.scalar.dma_start(out=trilb_sb, in_=trilb_d[...])
        eye_sb = perm.tile([P, KT], BF16)
        nc.scalar.dma_start(out=eye_sb, in_=eye_d[...])
        bqk_sb = perm.tile([P, 4], F32)
        nc.scalar.dma_start(out=bqk_sb, in_=bqk_d[...].rearrange("(jt p) -> p jt", p=P))
        bv_sb = perm.tile([P, 256], F32)
        nc.scalar.dma_start(
            out=bv_sb, in_=bass.AP(tensor=bv_d, offset=0, ap=[[0, P], [1, 256]])
        )
        wp_sb = perm.tile([P, 2, D], F32R)
        nc.sync.dma_start(
            out=wp_sb, in_=wp_d[...].rearrange("(ft p) e -> p ft e", p=P)
        )

        # first Exp triggers a ~2.7us ACT table load; fire it early on the
        # tiny tril const so it doesn't sit on the phase-1->2 critical chain
        expwarm = perm.tile([P, 8], F32)
        nc.scalar.activation(out=expwarm, in_=tril_sb[:, 0:8], func=Exp)

        qk_sb = perm.tile([P, 4, T], BF16)        # jt 0,1: Q^T; jt 2,3: K^T
        vaug_sb = perm.tile([P, 16, HL, 128], F32R)  # [kv, tt, h, V(64)|ones(64)]
        vaug_hh = vaug_sb.rearrange("p t (hp hh) c -> p t hp hh c", hh=2)
        ctxn_sb = perm.tile([P, 2, T], F32R)      # normalized ctx^T

        # the ones halves of the [V|ones]/[ones|V] stationaries are constant:
        # write them once up front (memset can't emit f32r, so write the f32
        # bit pattern of 1.0 through a uint32 view)
        ONE_F32 = 0x3F800000
        nc.gpsimd.memset(
            vaug_hh[:, :, :, 0, 64:128].bitcast(mybir.dt.uint32), ONE_F32
        )
        nc.gpsimd.memset(
            vaug_hh[:, :, :, 1, 0:64].bitcast(mybir.dt.uint32), ONE_F32
        )

        # --- phase-1 building blocks (emitted interleaved with attention) ---
        def qk_group(xq, tt4, jt):
            ps = psC.tile([P, QT], F32, name=f"qkps{tt4}_{jt}", tag="C")
            for dt in range(8):
                nc.tensor.matmul(
                    out=ps,
                    lhsT=wqk_sb[:, dt, jt * 128:(jt + 1) * 128],
                    rhs=xq[:, dt, :],
                    start=(dt == 0),
                    stop=(dt == 7),
                )
            nc.scalar.activation(
                out=qk_sb[:, jt, tt4 * QT:(tt4 + 1) * QT],
                in_=ps,
                func=Ident,
                bias=bqk_sb[:, jt:jt + 1],
                scale=1.0,
            )

        def v_group(xq, tt4, k):
            tt = tt4 * 4 + k
            psv = psA.tile([P, 256], F32, name=f"vps{tt}", tag="acc")
            for dt in range(8):
                nc.tensor.matmul(
                    out=psv,
                    lhsT=xq[:, dt, k * 128:(k + 1) * 128],
                    rhs=wv_sb[:, dt, :],
                    start=(dt == 0),
                    stop=(dt == 7),
                )
            # even heads fill [V|ones], odd heads fill [ones|V]; the flip
            # keeps ctx rows lane-aligned with the ctxn_sb head-pair packing
            vview = vaug_hh[:, tt]
            pview = psv.rearrange("p (hp hh d) -> p hp hh d", hp=2, hh=2)
            bview = bv_sb.rearrange("p (hp hh d) -> p hp hh d", hp=2, hh=2)
            nc.vector.tensor_add(
                out=vview[:, :, 0, 0:64], in0=pview[:, :, 0, :], in1=bview[:, :, 0, :]
            )
            nc.vector.tensor_add(
                out=vview[:, :, 1, 64:128], in0=pview[:, :, 1, :], in1=bview[:, :, 1, :]
            )

        DEPTH = 4
        Th_by_blk = {}

        def qoff_of(qi, j):
            return max(0, KT * j - qi * QT)

        def stage_a(hp, qi, j):
            q0 = qi * QT
            qoff = qoff_of(qi, j)
            masked = j >= 4 * qi
            s = psA.tile([P, 2 * QT], F32, name=f"s{hp}_{qi}_{j}", tag="acc")
            for hh in range(2):
                nc.tensor.matmul(
                    out=s[:, hh * QT + qoff: hh * QT + QT],
                    lhsT=qk_sb[hh * 64:(hh + 1) * 64, 2 + hp, j * KT:(j + 1) * KT],
                    rhs=qk_sb[hh * 64:(hh + 1) * 64, hp, q0 + qoff:q0 + QT],
                    start=True,
                    stop=not masked,
                )
            if masked:
                # diagonal tile: accumulate the -1e30 tril constant into the
                # 128-wide triangular sub-block ON THE PE (identity
                # stationary, broadcast moving) — keeps the S -> exp chain
                # off the DVE queue entirely
                sm = s.rearrange("p (hh c) -> p hh c", hh=2)[:, :, qoff:qoff + KT]
                mask_b = bass.AP(
                    tensor=trilb_sb.tensor,
                    offset=trilb_sb.offset,
                    ap=[trilb_sb.ap[0], [0, 2], [1, KT]],
                )
                nc.tensor.matmul(
                    out=sm, lhsT=eye_sb, rhs=mask_b, start=False, stop=True,
                )
            p_t = ppool.tile([P, 2, QT], F32R, name=f"p{hp}_{qi}_{j}", tag="p")
            sv = s.rearrange("p (hh c) -> p hh c", hh=2)
            nc.scalar.activation(
                out=p_t[:, :, qoff:QT], in_=sv[:, :, qoff:QT], func=Exp
            )
            return p_t

        def stage_b(hp, qi, j, p_t):
            njt = 4 * qi + 4
            qoff = qoff_of(qi, j)
            if j == 0:
                Th_by_blk[(hp, qi)] = [
                    psC.tile([P, QT], F32, name=f"T{hp}_{qi}_{hh}", tag="C")
                    for hh in range(2)
                ]
            Th = Th_by_blk[(hp, qi)]
            for hh in range(2):
                nc.tensor.matmul(
                    out=Th[hh][:, qoff:QT],
                    lhsT=vaug_sb[:, j, hp * 2 + hh, :],
                    rhs=p_t[:, hh, qoff:QT],
                    start=(j == 0),
                    stop=(j == njt - 1),
                )
            if j == njt - 1:
                normalize(hp, qi)

        def normalize(hp, qi):
            q0 = qi * QT
            Th = Th_by_blk.pop((hp, qi))
            for hh in range(2):
                cl = hh * 64          # ctx lanes base
                rec = rpool.tile([P, QT], F32, name=f"rec{hp}_{qi}_{hh}", tag="rec")
                # reciprocal_approx_fast mis-executes at partition base 64
                # (HW-verified), so always run it at base 0.
                if hh == 1:
                    nc.vector.reciprocal_approx_fast(out=rec[0:64, :], in_=Th[hh][0:64, :])
                    nc.sync.dma_start(out=rec[64:128, :], in_=rec[0:64, :])
                else:
                    # the den-row copy rides ACT (DVE is the scarcer engine
                    # in the normalize chain)
                    nc.scalar.activation(
                        out=rec[64:128, :], in_=Th[hh][64:128, :], func=Ident
                    )
                    nc.sync.dma_start(out=rec[0:64, :], in_=rec[64:128, :])
                    nc.vector.reciprocal_approx_fast(out=rec[0:64, :], in_=rec[0:64, :])
                nc.vector.tensor_mul(
                    out=ctxn_sb[cl:cl + 64, hp, q0:q0 + QT],
                    in0=Th[hh][cl:cl + 64, :],
                    in1=rec[cl:cl + 64, :],
                )

        def proj_block(tt):
            pj = psA.tile([P, 2, QT], F32, name=f"pj{tt}", tag="acc")
            ob = opool.tile([P, D], BF16, name=f"ob{tt}", tag="ob")
            for et in range(2):
                for ft in range(2):
                    nc.tensor.matmul(
                        out=pj[:, et, :],
                        lhsT=ctxn_sb[:, ft, tt * KT:(tt + 1) * KT],
                        rhs=wp_sb[:, ft, et * QT:(et + 1) * QT],
                        start=(ft == 0),
                        stop=(ft == 1),
                    )
            # PSUM->SBUF cast on DVE (ACT is the scarcer engine; GPSIMD
            # can't read PSUM)
            nc.vector.tensor_copy(out=ob, in_=pj.rearrange("p a b -> p (a b)"))
            nc.sync.dma_start(out=out_d[tt * KT:(tt + 1) * KT, :], in_=ob)

        # --- unified slot scheduler: qkv quarter q, then attention blocks
        # (q, hp) whose kv-tiles only need quarters <= q. Every PE work item
        # is a "slot"; pending ctx matmuls (stage_b) trail stage_a by DEPTH
        # slots and drain at 2/slot across block boundaries; proj blocks pop
        # a few slots after their row-block's last normalize. ---
        pend = deque()
        projq = deque()
        normed = {}
        slot = 0

        def emit_b():
            hp, qi, j, p_t = pend.popleft()
            stage_b(hp, qi, j, p_t)
            if j == 4 * qi + 3:
                normed[qi] = normed.get(qi, 0) + 1
                if normed[qi] == 2:
                    projq.extend((tt, slot) for tt in range(4 * qi, 4 * qi + 4))

        def tick(cur_block, reserve=0):
            nonlocal slot
            if pend and (pend[0][0], pend[0][1]) != cur_block:
                emit_b()
                if pend and (pend[0][0], pend[0][1]) != cur_block:
                    emit_b()
            elif len(pend) > DEPTH:
                emit_b()
            # proj blocks trail their qi's last normalize by >=4 slots so the
            # DVE reciprocal chain finishes before the PE reaches them
            if (
                len(projq) > reserve
                and slot % 2 == 0
                and slot - projq[0][1] >= 4
            ):
                proj_block(projq.popleft()[0])
            slot += 1

        xq_tiles = {0: xq0}
        NQ = T // QT
        for q in range(NQ):
            # prefetch the NEXT quarter's x; quarter 1 rides the fast ACT
            # HWDGE queue (the gpsimd SWDGE ring takes ~7us to boot), the
            # rest go on gpsimd which is idle by then
            if q + 1 < NQ:
                nxt = xpool.tile([P, 8, QT], BF16, name=f"xq{q + 1}", tag="xq")
                dma_eng = nc.scalar if q == 0 else nc.gpsimd
                dma_eng.dma_start(
                    out=nxt,
                    in_=xT_d[:, (q + 1) * QT:(q + 2) * QT].rearrange(
                        "(dt p) t -> p dt t", p=P
                    ),
                )
                xq_tiles[q + 1] = nxt
            xq = xq_tiles.pop(q)
            for jt in range(4):
                qk_group(xq, q, jt)
                tick(None)
            for k in range(4):
                v_group(xq, q, k)
                tick(None)
            for hp in range(2):
                # in the very last block, hold a couple of proj blocks back
                # so the final normalize chain has PE work to hide behind
                reserve = 2 if (q == NQ - 1 and hp == 1) else 0
                for j in range(4 * q + 4):
                    p_t = stage_a(hp, q, j)
                    pend.append((hp, q, j, p_t))
                    tick((hp, q), reserve)
        while pend:
            emit_b()
        while projq:
            proj_block(projq.popleft()[0])

    nc.finalize()
    return nc


_NC_CACHE: list = []


def _get_nc() -> bass.Bass:
    if not _NC_CACHE:
        _NC_CACHE.append(_build_nc())
    return _NC_CACHE[0]


def _shard_inputs(x, W_qkv, b_qkv, W_proj):
    scale = np.float32(1.0 / np.sqrt(DH))
    bf16 = ml_dtypes.bfloat16
    in_maps = []
    xTs = [np.ascontiguousarray(x[b].T.astype(bf16)) for b in range(B)]
    for c in range(NCORES):
        b = c // 4
        lo = (c % 4) * HL * DH
        wqk = np.concatenate(
            [W_qkv[:, lo:lo + 256] * scale, W_qkv[:, D + lo:D + lo + 256]], axis=1
        )
        bqk = np.concatenate([b_qkv[lo:lo + 256] * scale, b_qkv[D + lo:D + lo + 256]])
        in_maps.append({
            "xT": xTs[b],
            "wqk": np.ascontiguousarray(wqk.astype(bf16)),
            "wv": np.ascontiguousarray(W_qkv[:, 2 * D + lo:2 * D + lo + 256].astype(bf16)),
            "bqk": np.ascontiguousarray(bqk, np.float32),
            "bv": np.ascontiguousarray(b_qkv[2 * D + lo:2 * D + lo + 256], np.float32),
            "wp": np.ascontiguousarray(W_proj[lo:lo + 256, :], np.float32),
        })
    return in_maps


def kernel(x, W_qkv, b_qkv, W_proj, b_proj, _trace=False, _tmpdir=None):
    x = np.asarray(x, np.float32)
    W_qkv = np.asarray(W_qkv, np.float32)
    b_qkv = np.asarray(b_qkv, np.float32)
    W_proj = np.asarray(W_proj, np.float32)
    b_proj = np.asarray(b_proj, np.float32)

    nc = _get_nc()
    in_maps = _shard_inputs(x, W_qkv, b_qkv, W_proj)
    kw = {}
    if _trace:
        kw = dict(trace=True, tmpdir=_tmpdir)
    r = run_bass_kernel_spmd(nc, in_maps, core_ids=list(range(NCORES)), **kw)
    out = np.zeros((B, T, D), np.float32)
    for c in range(NCORES):
        out[c // 4] += np.asarray(r.results[c]["out"], np.float32)
    out += b_proj[None, None, :]
    if _trace:
        return out, r
    return out


# revision 43
# speedup vs baseline: 1.1073x; 1.0294x over previous
"""Multi-head self-attention (B=2, T=2048, D=1024, H=16) on 8 TRN2 NeuronCores.

Sharding: batch x head-group. Core c handles batch b = c//4 and heads
h0 = 4*(c%4) .. h0+4 (Megatron-style column split of W_qkv, row split of
W_proj). Each core computes qkv projection for its heads, causal
flash-style attention for its 4 heads, and a partial output projection;
the host sums the 4 partial projections per batch (the Megatron
all-reduce realized as the unshard step) and adds b_proj.

Device algorithm (per core; all matmul operands bf16 -> 1 cyc/row at any
N, f32 PSUM accumulation):
  - qk^T[j, t] = sum_d W[d, j] x[t, d]  (j on partitions -> Q^T, K^T)
  - v[t, j]    = sum_d x[t, d] Wv[d, j] (t on partitions), packed into
    per-head [V_h|ones] / [ones|V_h] bf16 stationaries (parity-flipped so
    ctx lands on the lanes the ctxn head-pair packing needs)
  - S^T[kv, q] = K^T_tile.T @ Q^T, two heads row-packed per 2-bank PSUM
    pair; causal sub-diagonal tiles are skipped entirely, diagonal blocks
    masked by one strided DVE add of an inline tril constant
  - P^T = exp(S^T): one fused 2-head ScalarE activation per kv-tile,
    written bf16 (2x ACT throughput; softmax scale pre-folded into W_q)
  - one [ctx|den] matmul per (head, kv-tile) accumulates context and the
    softmax denominators together (denominator rows come from the ones
    half of the stationary, so they cost no extra PE cycles)
  - ctxn^T = ctx * reciprocal(den): reciprocal_approx_fast must run at
    partition base 0 (HW bug at base 64), so the recip rows are
    lane-shifted to the ctx lanes with a small SBUF->SBUF DMA
  - out_partial[t, e] = sum_f ctxn^T[f, t] Wp[f, e], written as bf16
    partials (host sums partials in fp32)

Scheduling: startup DMAs are split per-d-tile so the first qkv matmul
issues ~2us in (vs waiting for all weights); phases 2+3 run as ONE
global software pipeline: stage_b (ctx matmuls) trail stage_a (S^T +
exp) by DEPTH kv-tiles across ALL (qi, head-pair) block boundaries (no
drains), with the output-projection blocks of row-block qi interleaved
into the next block's stream. The PE queue therefore never idles, which
also keeps the tensor engine at its max p-state clock.
"""

import sys

if "/opt/trn_rl_repo" not in sys.path:
    sys.path.insert(0, "/opt/trn_rl_repo")

from collections import deque
from contextlib import ExitStack

import ml_dtypes
import numpy as np

import concourse.bass as bass
import concourse.bacc as bacc
import concourse.tile as tile
from concourse import mybir
from concourse.bass_utils import run_bass_kernel_spmd

B, T, D, H, DH = 2, 2048, 1024, 16, 64
NCORES = 8
HL = 4          # heads per core
P = 128         # SBUF/PSUM partitions
QT = 512        # q tile (moving free dim / PSUM bank)
KT = 128        # kv tile (PSUM partition dim)
F32 = mybir.dt.float32
F32R = mybir.dt.float32r
BF16 = mybir.dt.bfloat16


def _build_nc() -> bass.Bass:
    nc = bacc.Bacc(None)
    Exp = mybir.ActivationFunctionType.Exp
    Ident = mybir.ActivationFunctionType.Identity

    xT_d = nc.dram_tensor("xT", [D, T], BF16, kind="ExternalInput")
    wqk_d = nc.dram_tensor("wqk", [D, 512], BF16, kind="ExternalInput")
    wv_d = nc.dram_tensor("wv", [D, 256], BF16, kind="ExternalInput")
    bqk_d = nc.dram_tensor("bqk", [512], F32, kind="ExternalInput")
    bv_d = nc.dram_tensor("bv", [256], F32, kind="ExternalInput")
    wp_d = nc.dram_tensor("wp", [256, D], F32R, kind="ExternalInput")
    out_d = nc.dram_tensor("out", [T, D], BF16, kind="ExternalOutput")

    kv = np.arange(KT)
    tril_np = np.where(kv[:, None] <= kv[None, :], 0.0, -1e30).astype(np.float32)
    tril_d = nc.inline_tensor(tril_np, name="tril")
    trilb_np = tril_np.astype(ml_dtypes.bfloat16)
    trilb_d = nc.inline_tensor(trilb_np, name="trilb")
    eye_np = np.eye(KT, dtype=ml_dtypes.bfloat16)
    eye_d = nc.inline_tensor(eye_np, name="eye")

    with tile.TileContext(nc) as tc, ExitStack() as ctx:
        perm = ctx.enter_context(tc.tile_pool(name="perm", bufs=1))
        xpool = ctx.enter_context(tc.tile_pool(name="xpool", bufs=4))
        ppool = ctx.enter_context(tc.tile_pool(name="ppool", bufs=6))
        rpool = ctx.enter_context(tc.tile_pool(name="rpool", bufs=3))
        opool = ctx.enter_context(tc.tile_pool(name="opool", bufs=3))
        psA = ctx.enter_context(tc.tile_pool(name="psA", bufs=2, space="PSUM"))
        psC = ctx.enter_context(tc.tile_pool(name="psC", bufs=4, space="PSUM"))

        # --- the first x quarter rides the ACT HWDGE queue, split in chunks,
        # ahead of everything else on that queue, so the first qkv matmul can
        # issue ~2us in; later quarters prefetch on the (slow, idle) gpsimd
        # SWDGE queue ---
        xq0 = xpool.tile([P, 8, QT], BF16, name="xq0", tag="xq")
        for lo, hi in ((0, 1), (1, 2), (2, 4), (4, 6), (6, 8)):
            nc.scalar.dma_start(
                out=xq0[:, lo:hi, :],
                in_=xT_d[128 * lo:128 * hi, 0:QT].rearrange(
                    "(dt p) t -> p dt t", p=P
                ),
            )

        # --- weights / constants; DMAs split so the first matmul can start
        # as soon as the first (wqk, xq) chunk pair lands ---
        wqk_sb = perm.tile([P, 8, 512], BF16)
        wv_sb = perm.tile([P, 8, 256], BF16)

        def wqk_chunk(lo, hi):
            nc.sync.dma_start(
                out=wqk_sb[:, lo:hi, :],
                in_=wqk_d[128 * lo:128 * hi, :].rearrange(
                    "(dt p) j -> p dt j", p=P
                ),
            )

        def wv_chunk(lo, hi):
            nc.sync.dma_start(
                out=wv_sb[:, lo:hi, :],
                in_=wv_d[128 * lo:128 * hi, :].rearrange(
                    "(dt p) j -> p dt j", p=P
                ),
            )

        # interleaved so the V groups (which start ~7us in) aren't starved
        # behind the full wqk load
        wqk_chunk(0, 1)
        wqk_chunk(1, 2)
        wqk_chunk(2, 4)
        wv_chunk(0, 2)
        wqk_chunk(4, 6)
        wv_chunk(2, 4)
        wqk_chunk(6, 8)
        wv_chunk(4, 6)
        wv_chunk(6, 8)
        # tril + bias consts ride the ACT queue so expwarm (which needs
        # tril) runs immediately and never blocks the phase-1 bias adds
        tril_sb = perm.tile([P, KT], F32)
        nc.scalar.dma_start(out=tril_sb, in_=tril_d[...])
        trilb_sb = perm.tile([P, KT], BF16)
        nc.scalar.dma_start(out=trilb_sb, in_=trilb_d[...])
        eye_sb = perm.tile([P, KT], BF16)
        nc.scalar.dma_start(out=eye_sb, in_=eye_d[...])
        bqk_sb = perm.tile([P, 4], F32)
        nc.scalar.dma_start(out=bqk_sb, in_=bqk_d[...].rearrange("(jt p) -> p jt", p=P))
        bv_sb = perm.tile([P, 256], F32)
        nc.scalar.dma_start(
            out=bv_sb, in_=bass.AP(tensor=bv_d, offset=0, ap=[[0, P], [1, 256]])
        )
        wp_sb = perm.tile([P, 2, D], F32R)
        nc.sync.dma_start(
            out=wp_sb, in_=wp_d[...].rearrange("(ft p) e -> p ft e", p=P)
        )

        # first Exp triggers a ~2.7us ACT table load; fire it early on the
        # tiny tril const so it doesn't sit on the phase-1->2 critical chain
        expwarm = perm.tile([P, 8], F32)
        nc.scalar.activation(out=expwarm, in_=tril_sb[:, 0:8], func=Exp)

        qk_sb = perm.tile([P, 4, T], BF16)        # jt 0,1: Q^T; jt 2,3: K^T
        vaug_sb = perm.tile([P, 16, HL, 128], F32R)  # [kv, tt, h, V(64)|ones(64)]
        vaug_hh = vaug_sb.rearrange("p t (hp hh) c -> p t hp hh c", hh=2)
        ctxn_sb = perm.tile([P, 2, T], F32R)      # normalized ctx^T

        # the ones halves of the [V|ones]/[ones|V] stationaries are constant:
        # write them once up front (memset can't emit f32r, so write the f32
        # bit pattern of 1.0 through a uint32 view)
        ONE_F32 = 0x3F800000
        nc.gpsimd.memset(
            vaug_hh[:, :, :, 0, 64:128].bitcast(mybir.dt.uint32), ONE_F32
        )
        nc.gpsimd.memset(
            vaug_hh[:, :, :, 1, 0:64].bitcast(mybir.dt.uint32), ONE_F32
        )

        # --- phase-1 building blocks (emitted interleaved with attention) ---
        def qk_group(xq, tt4, jt):
            ps = psC.tile([P, QT], F32, name=f"qkps{tt4}_{jt}", tag="C")
            for dt in range(8):
                nc.tensor.matmul(
                    out=ps,
                    lhsT=wqk_sb[:, dt, jt * 128:(jt + 1) * 128],
                    rhs=xq[:, dt, :],
                    start=(dt == 0),
                    stop=(dt == 7),
                )
            nc.scalar.activation(
                out=qk_sb[:, jt, tt4 * QT:(tt4 + 1) * QT],
                in_=ps,
                func=Ident,
                bias=bqk_sb[:, jt:jt + 1],
                scale=1.0,
            )

        def v_group(xq, tt4, kp):
            # one psum tile covers a PAIR of kv-tiles: halves the number of
            # V-pack DVE ops and doubles the psum-rotation slack
            tt = tt4 * 4 + 2 * kp
            psv = psA.tile([P, 2, 256], F32, name=f"vps{tt}", tag="acc")
            for kk in range(2):
                for dt in range(8):
                    nc.tensor.matmul(
                        out=psv[:, kk, :],
                        lhsT=xq[:, dt, (2 * kp + kk) * 128:(2 * kp + kk + 1) * 128],
                        rhs=wv_sb[:, dt, :],
                        start=(dt == 0),
                        stop=(dt == 7),
                    )
            # even heads fill [V|ones], odd heads fill [ones|V]; the flip
            # keeps ctx rows lane-aligned with the ctxn_sb head-pair packing
            vview = vaug_hh[:, tt:tt + 2]
            pview = psv.rearrange("p t (hp hh d) -> p t hp hh d", hp=2, hh=2)
            bv4 = bv_sb.rearrange("p (hp hh d) -> p hp hh d", hp=2, hh=2)

            def bvb(hh_):
                # bias slice [p, hp, 64] broadcast across the kv-tile pair
                v = bv4[:, :, hh_, :]
                return bass.AP(
                    tensor=v.tensor, offset=v.offset,
                    ap=[v.ap[0], [0, 2]] + list(v.ap[1:]),
                )

            nc.vector.tensor_add(
                out=vview[:, :, :, 0, 0:64], in0=pview[:, :, :, 0, :], in1=bvb(0)
            )
            nc.vector.tensor_add(
                out=vview[:, :, :, 1, 64:128], in0=pview[:, :, :, 1, :], in1=bvb(1)
            )

        DEPTH = 4
        Th_by_blk = {}

        def qoff_of(qi, j):
            return max(0, KT * j - qi * QT)

        def stage_a(hp, qi, j):
            q0 = qi * QT
            qoff = qoff_of(qi, j)
            masked = j >= 4 * qi
            s = psA.tile([P, 2 * QT], F32, name=f"s{hp}_{qi}_{j}", tag="acc")
            for hh in range(2):
                nc.tensor.matmul(
                    out=s[:, hh * QT + qoff: hh * QT + QT],
                    lhsT=qk_sb[hh * 64:(hh + 1) * 64, 2 + hp, j * KT:(j + 1) * KT],
                    rhs=qk_sb[hh * 64:(hh + 1) * 64, hp, q0 + qoff:q0 + QT],
                    start=True,
                    stop=not masked,
                )
            if masked:
                # diagonal tile: accumulate the -1e30 tril constant into the
                # 128-wide triangular sub-block ON THE PE (identity
                # stationary, broadcast moving) — keeps the S -> exp chain
                # off the DVE queue entirely
                sm = s.rearrange("p (hh c) -> p hh c", hh=2)[:, :, qoff:qoff + KT]
                mask_b = bass.AP(
                    tensor=trilb_sb.tensor,
                    offset=trilb_sb.offset,
                    ap=[trilb_sb.ap[0], [0, 2], [1, KT]],
                )
                nc.tensor.matmul(
                    out=sm, lhsT=eye_sb, rhs=mask_b, start=False, stop=True,
                )
            p_t = ppool.tile([P, 2, QT], F32R, name=f"p{hp}_{qi}_{j}", tag="p")
            sv = s.rearrange("p (hh c) -> p hh c", hh=2)
            nc.scalar.activation(
                out=p_t[:, :, qoff:QT], in_=sv[:, :, qoff:QT], func=Exp
            )
            return p_t

        def stage_b(hp, qi, j, p_t):
            njt = 4 * qi + 4
            qoff = qoff_of(qi, j)
            if j == 0:
                Th_by_blk[(hp, qi)] = [
                    psC.tile([P, QT], F32, name=f"T{hp}_{qi}_{hh}", tag="C")
                    for hh in range(2)
                ]
            Th = Th_by_blk[(hp, qi)]
            for hh in range(2):
                nc.tensor.matmul(
                    out=Th[hh][:, qoff:QT],
                    lhsT=vaug_sb[:, j, hp * 2 + hh, :],
                    rhs=p_t[:, hh, qoff:QT],
                    start=(j == 0),
                    stop=(j == njt - 1),
                )
            if j == njt - 1:
                normalize(hp, qi)

        def normalize(hp, qi):
            q0 = qi * QT
            Th = Th_by_blk.pop((hp, qi))
            for hh in range(2):
                cl = hh * 64          # ctx lanes base
                rec = rpool.tile([P, QT], F32, name=f"rec{hp}_{qi}_{hh}", tag="rec")
                # reciprocal_approx_fast mis-executes at partition base 64
                # (HW-verified), so always run it at base 0.
                if hh == 1:
                    nc.vector.reciprocal_approx_fast(out=rec[0:64, :], in_=Th[hh][0:64, :])
                    nc.sync.dma_start(out=rec[64:128, :], in_=rec[0:64, :])
                else:
                    # the den-row copy rides ACT (DVE is the scarcer engine
                    # in the normalize chain)
                    nc.scalar.activation(
                        out=rec[64:128, :], in_=Th[hh][64:128, :], func=Ident
                    )
                    nc.sync.dma_start(out=rec[0:64, :], in_=rec[64:128, :])
                    nc.vector.reciprocal_approx_fast(out=rec[0:64, :], in_=rec[0:64, :])
                nc.vector.tensor_mul(
                    out=ctxn_sb[cl:cl + 64, hp, q0:q0 + QT],
                    in0=Th[hh][cl:cl + 64, :],
                    in1=rec[cl:cl + 64, :],
                )

        def proj_block(tt):
            pj = psA.tile([P, 2, QT], F32, name=f"pj{tt}", tag="acc")
            ob = opool.tile([P, D], BF16, name=f"ob{tt}", tag="ob")
            for et in range(2):
                for ft in range(2):
                    nc.tensor.matmul(
                        out=pj[:, et, :],
                        lhsT=ctxn_sb[:, ft, tt * KT:(tt + 1) * KT],
                        rhs=wp_sb[:, ft, et * QT:(et + 1) * QT],
                        start=(ft == 0),
                        stop=(ft == 1),
                    )
            # PSUM->SBUF cast on DVE (ACT is the scarcer engine; GPSIMD
            # can't read PSUM)
            nc.vector.tensor_copy(out=ob, in_=pj.rearrange("p a b -> p (a b)"))
            nc.sync.dma_start(out=out_d[tt * KT:(tt + 1) * KT, :], in_=ob)

        # --- unified slot scheduler: qkv quarter q, then attention blocks
        # (q, hp) whose kv-tiles only need quarters <= q. Every PE work item
        # is a "slot"; pending ctx matmuls (stage_b) trail stage_a by DEPTH
        # slots and drain at 2/slot across block boundaries; proj blocks pop
        # a few slots after their row-block's last normalize. ---
        pend = deque()
        projq = deque()
        normed = {}
        slot = 0

        def emit_b():
            hp, qi, j, p_t = pend.popleft()
            stage_b(hp, qi, j, p_t)
            if j == 4 * qi + 3:
                normed[qi] = normed.get(qi, 0) + 1
                if normed[qi] == 2:
                    projq.extend((tt, slot) for tt in range(4 * qi, 4 * qi + 4))

        def tick(cur_block, reserve=0):
            nonlocal slot
            if pend and (pend[0][0], pend[0][1]) != cur_block:
                emit_b()
                if pend and (pend[0][0], pend[0][1]) != cur_block:
                    emit_b()
            elif len(pend) > DEPTH:
                emit_b()
            # proj blocks trail their qi's last normalize by >=4 slots so the
            # DVE reciprocal chain finishes before the PE reaches them
            if (
                len(projq) > reserve
                and slot % 2 == 0
                and slot - projq[0][1] >= 6
            ):
                proj_block(projq.popleft()[0])
            slot += 1

        xq_tiles = {0: xq0}
        NQ = T // QT
        for q in range(NQ):
            # prefetch the NEXT quarter's x; quarter 1 rides the fast ACT
            # HWDGE queue (the gpsimd SWDGE ring takes ~7us to boot), the
            # rest go on gpsimd which is idle by then
            if q + 1 < NQ:
                nxt = xpool.tile([P, 8, QT], BF16, name=f"xq{q + 1}", tag="xq")
                dma_eng = nc.scalar if q == 0 else nc.gpsimd
                dma_eng.dma_start(
                    out=nxt,
                    in_=xT_d[:, (q + 1) * QT:(q + 2) * QT].rearrange(
                        "(dt p) t -> p dt t", p=P
                    ),
                )
                xq_tiles[q + 1] = nxt
            xq = xq_tiles.pop(q)
            for jt in range(4):
                qk_group(xq, q, jt)
                tick(None)
            for kp in range(2):
                v_group(xq, q, kp)
                tick(None)
            for hp in range(2):
                # during the last quarter's blocks, hold the remaining proj
                # blocks back so the final normalize chain has PE work to
                # hide behind at the drain
                reserve = 4 if q == NQ - 1 else 0
                for j in range(4 * q + 4):
                    p_t = stage_a(hp, q, j)
                    pend.append((hp, q, j, p_t))
                    tick((hp, q), reserve)
        while pend:
            emit_b()
            if projq:
                proj_block(projq.popleft()[0])
        while projq:
            proj_block(projq.popleft()[0])

    nc.finalize()
    return nc


_NC_CACHE: list = []


def _get_nc() -> bass.Bass:
    if not _NC_CACHE:
        _NC_CACHE.append(_build_nc())
    return _NC_CACHE[0]


def _shard_inputs(x, W_qkv, b_qkv, W_proj):
    scale = np.float32(1.0 / np.sqrt(DH))
    bf16 = ml_dtypes.bfloat16
    in_maps = []
    xTs = [np.ascontiguousarray(x[b].T.astype(bf16)) for b in range(B)]
    for c in range(NCORES):
        b = c // 4
        lo = (c % 4) * HL * DH
        wqk = np.concatenate(
            [W_qkv[:, lo:lo + 256] * scale, W_qkv[:, D + lo:D + lo + 256]], axis=1
        )
        bqk = np.concatenate([b_qkv[lo:lo + 256] * scale, b_qkv[D + lo:D + lo + 256]])
        in_maps.append({
            "xT": xTs[b],
            "wqk": np.ascontiguousarray(wqk.astype(bf16)),
            "wv": np.ascontiguousarray(W_qkv[:, 2 * D + lo:2 * D + lo + 256].astype(bf16)),
            "bqk": np.ascontiguousarray(bqk, np.float32),
            "bv": np.ascontiguousarray(b_qkv[2 * D + lo:2 * D + lo + 256], np.float32),
            "wp": np.ascontiguousarray(W_proj[lo:lo + 256, :], np.float32),
        })
    return in_maps


def kernel(x, W_qkv, b_qkv, W_proj, b_proj, _trace=False, _tmpdir=None):
    x = np.asarray(x, np.float32)
    W_qkv = np.asarray(W_qkv, np.float32)
    b_qkv = np.asarray(b_qkv, np.float32)
    W_proj = np.asarray(W_proj, np.float32)
    b_proj = np.asarray(b_proj, np.float32)

    nc = _get_nc()
    in_maps = _shard_inputs(x, W_qkv, b_qkv, W_proj)
    kw = {}
    if _trace:
        kw = dict(trace=True, tmpdir=_tmpdir)
    r = run_bass_kernel_spmd(nc, in_maps, core_ids=list(range(NCORES)), **kw)
    out = np.zeros((B, T, D), np.float32)
    for c in range(NCORES):
        out[c // 4] += np.asarray(r.results[c]["out"], np.float32)
    out += b_proj[None, None, :]
    if _trace:
        return out, r
    return out
